# revision 22
# baseline (speedup 1.0000x reference)
"""Trainium2 kernel for nn_Graph_41609643163904.

The reference op is a sequential per-cell scatter sweep over a 48x48 grid
(x outer, y inner): read center v, zero it, add v*W[y,x] to the 5x5
neighborhood.  Every step is linear in the grid, so the sweep is a fixed
linear operator M (2304x2304) of the weights; the baseline ran the full
block-banded M as 188 dense 128x128 matmul blocks.

v3 exploits the sweep's sequentially-semiseparable (SSS) structure: all
influence crossing grid-column x flows through the 88-dim interface
s(x) = [v(x); v(x-1)] (v = per-cell fired values, 44 per column).  In
x-major layout, each 128-row output tile j (covering grid columns
jc_lo..jc_hi) decomposes EXACTLY as

    out_j = sum_k  nearblk[j,k] @ g0_ktile[k]   (k over cols >= jc_lo: 2-3 tiles)
          + U_j @ s(jc_lo - 1)                  (everything to the left)

and the 16 needed states are propagated by a 16-step chain
    s_i = W_i.T @ [s_{i-1}; g0_interior(cols between cuts)]
with K<=128 per matmul.  Total: 47 near + 16 far + 33 chain lhsT blocks
= 95 logical matmuls vs the baseline's 188 dense blocks.  All operands
bf16 (tolerance is 2e-2; bf16 end-to-end lands ~5e-3), which also halves
DMA.  Data-parallel over the 8192-sample batch across 8 cores, no comm.

Device flow per core: x k-tiles + operator stream + interior-column
gathers (SBUF->SBUF) feed a single PE stream: near(j0), near(j1), then
[chain_i -> out(j=i+2)] interleaved so state copies hide under the near
matmuls of the following output tile.
"""

import os

import numpy as np

SIZE = 48
D = 2
KS = 5
N = SIZE * SIZE          # 2304
B = 8192
NCORES = 8
BS = B // NCORES         # 1024 samples per core
P = 128
NT = N // P              # 18 tiles of 128
MW = 512                 # matmul moving-dim (PSUM bank)
NM = BS // MW            # 2 halves


# ---------------------------------------------------------------- plan ----

def _plan():
    js = []
    for j in range(NT):
        r0 = P * j
        jc_lo, jc_hi = r0 // SIZE, (r0 + P - 1) // SIZE
        ncol_lo = jc_lo if j >= 2 else 0
        ncol_hi = min(jc_hi + 2, SIZE - 1)
        kt_lo = (SIZE * ncol_lo) // P
        kt_hi = (SIZE * (ncol_hi + 1) - 1) // P
        js.append(dict(j=j, jc_lo=jc_lo, jc_hi=jc_hi, ncol_lo=ncol_lo,
                       kts=list(range(kt_lo, kt_hi + 1)),
                       cut=jc_lo - 1 if j >= 2 else None))
    cuts = sorted({d["cut"] for d in js if d["cut"] is not None})
    return js, cuts


_JS, _XS = _plan()
_NSTEP = len(_XS)  # 16 chain steps (step 0 = init)


def _step_cols(i):
    """grid columns consumed by chain step i"""
    c0 = 2 if i == 0 else _XS[i - 1] + 1
    return c0, _XS[i]


def _step_k(i):
    c0, c1 = _step_cols(i)
    ng = SIZE * (c1 - c0 + 1)          # FULL g0 columns (boundary rows get
    return ng if i == 0 else 88 + ng   # zero operator coeffs)


def _gather_segs(i):
    """DMA segments filling chain step i's rhs g0 rows (a contiguous
    x-major row range) from resident x k-tiles.
    Returns (dst: 'a'|'b', dst_p0, src_kt, src_p0, n)."""
    c0, c1 = _step_cols(i)
    a_rows = 128 if i == 0 else 40  # g0 rows living in tile_a (after state)
    a_base = 0 if i == 0 else 88
    g_lo, g_hi = SIZE * c0, SIZE * (c1 + 1)  # global row range
    segs = []
    r = 0
    g = g_lo
    while g < g_hi:
        kt = g // P
        n = min(g_hi - g, P * (kt + 1) - g)
        run = 0
        while run < n:
            if r + run < a_rows:
                m = min(n - run, a_rows - (r + run))
                segs.append(("a", a_base + r + run, kt, g - P * kt + run, m))
            else:
                m = n - run
                segs.append(("b", r + run - a_rows, kt, g - P * kt + run, m))
            run += m
        g += n
        r += n
    return segs


# ------------------------------------------------------- host operators ----

def _build_M_V(weights):
    """Composed operator M (N,N) and firing-value gradient rows V (1936,N),
    fp64, in the original y-major flattening."""
    M = np.eye(N, dtype=np.float64)
    V = np.zeros((44 * 44, N), dtype=np.float64)
    w = weights.astype(np.float64)
    for x in range(D, SIZE - D):
        for y in range(D, SIZE - D):
            c = y * SIZE + x
            wc = w[y, x]
            rc = M[c].copy()
            V[(x - D) * 44 + (y - D)] = rc
            for dy in range(-D, D + 1):
                r0 = c + dy * SIZE - D
                wrow = wc[dy + D]
                if dy == 0:
                    M[r0:r0 + D] += np.outer(wrow[:D], rc)
                    M[r0 + D + 1:r0 + KS] += np.outer(wrow[D + 1:], rc)
                else:
                    M[r0:r0 + KS] += np.outer(wrow, rc)
            M[c] = wc[D, D] * rc
    return M, V


def _xmajor_idx():
    n = np.arange(N)
    return (n % SIZE) * SIZE + n // SIZE


def _srows(X):
    return np.concatenate([(X - 2) * 44 + np.arange(44),
                           (X - 3) * 44 + np.arange(44)])


def _int_cols(c0, c1):
    return np.concatenate([SIZE * c + 2 + np.arange(44)
                           for c in range(c0, c1 + 1)])


def _build_operators(weights):
    M, V = _build_M_V(weights)
    idx = _xmajor_idx()
    Mx = M[np.ix_(idx, idx)]
    Vx = V[:, idx]
    ops = {}
    for d in _JS:
        j = d["j"]
        jr = slice(P * j, P * j + P)
        e_lo = SIZE * d["ncol_lo"]
        for kt in d["kts"]:
            blk = Mx[jr, P * kt:P * kt + P].copy()
            cols = np.arange(P * kt, P * kt + P)
            blk[:, cols < e_lo] = 0.0
            ops[("near", j, kt)] = blk.T        # lhsT (K=128, M=128)
        if d["cut"] is not None:
            X = d["cut"]
            sf = Vx[_srows(X), :SIZE * (X + 1)]
            F = Mx[jr, :SIZE * d["jc_lo"]]
            U, _, _, _ = np.linalg.lstsq(sf.T, F.T, rcond=None)
            ops[("far", j)] = U                 # lhsT (K=88, M=128)
    for i in range(_NSTEP):
        c0, c1 = _step_cols(i)
        X = _XS[i]
        Binj = Vx[_srows(X), SIZE * c0:SIZE * (c1 + 1)]  # full columns
        if i == 0:
            W = Binj.T
        else:
            Xp = _XS[i - 1]
            sf_p = Vx[_srows(Xp), :SIZE * (Xp + 1)]
            tgt = Vx[_srows(X), :SIZE * (Xp + 1)]
            T, _, _, _ = np.linalg.lstsq(sf_p.T, tgt.T, rcond=None)
            W = np.vstack([T, Binj.T])
        ops[("chain", i)] = W                   # lhsT (K_i, M=88)
    return ops


# ----------------------------------------------------- operator packing ----

def _pack_layout():
    """Column ranges in the packed wt tensor, in PE consumption order."""
    off = 0
    lay = {}

    def put(key, cols):
        nonlocal off
        lay[key] = (off, cols)
        off += cols

    for kt in _JS[0]["kts"]:
        put(("near", 0, kt), P)
    for kt in _JS[1]["kts"]:
        put(("near", 1, kt), P)
    for i in range(_NSTEP):
        put(("chain_a", i), 88)
        put(("chain_b", i), 88)
        j = i + 2
        for kt in _JS[j]["kts"]:
            put(("near", j, kt), P)
        put(("far", j), P)
    return lay, off


_LAY, _TOTC = _pack_layout()


def _pack_ops(ops):
    wt = np.zeros((P, _TOTC), dtype=np.float32)
    for d in _JS:
        j = d["j"]
        for kt in d["kts"]:
            o, c = _LAY[("near", j, kt)]
            wt[:, o:o + c] = ops[("near", j, kt)]
        if d["cut"] is not None:
            o, c = _LAY[("far", j)]
            wt[:88, o:o + P] = ops[("far", j)]
    for i in range(_NSTEP):
        W = ops[("chain", i)]
        Ktot = W.shape[0]
        o, _ = _LAY[("chain_a", i)]
        wt[:min(Ktot, P), o:o + 88] = W[:P]
        if Ktot > P:
            o, _ = _LAY[("chain_b", i)]
            wt[:Ktot - P, o:o + 88] = W[P:]
    return wt


# fetch groups: (group key list of layout keys) in consumption order
def _fetch_groups():
    gs = []
    gs.append(("near0", [("near", 0, kt) for kt in _JS[0]["kts"]]))
    gs.append(("near1", [("near", 1, kt) for kt in _JS[1]["kts"]]))
    for i in range(_NSTEP):
        gs.append((f"ch{i}", [("chain_a", i), ("chain_b", i)]))
        j = i + 2
        gs.append((f"out{j}",
                   [("near", j, kt) for kt in _JS[j]["kts"]] + [("far", j)]))
    return gs


_FETCH = _fetch_groups()
_WMAX = max(sum(_LAY[k][1] for k in keys) for _, keys in _FETCH)


# ------------------------------------------------------------- device ----

def _build_device_kernel():
    import concourse.mybir as mybir
    from concourse import bacc
    from concourse.tile import TileContext

    f32 = mybir.dt.float32
    bf16 = mybir.dt.bfloat16

    nc = bacc.Bacc()
    xT = nc.dram_tensor("xT", [N, BS], bf16, kind="ExternalInput")
    wt = nc.dram_tensor("wt", [P, _TOTC], bf16, kind="ExternalInput")
    outT = nc.dram_tensor("outT", [N, BS], bf16, kind="ExternalOutput")

    with TileContext(nc) as tc:
        with (
            tc.tile_pool(name="xpool", bufs=1) as xpool,
            tc.tile_pool(name="apool", bufs=1) as apool,
            tc.tile_pool(name="bpool", bufs=1) as bpool,
            tc.tile_pool(name="wpool", bufs=36) as wpool,
            tc.tile_pool(name="opool", bufs=3) as opool,
            tc.tile_pool(name="pso", bufs=2, space="PSUM") as pso,
            tc.tile_pool(name="pss", bufs=2, space="PSUM") as pss,
        ):
            # Engine / DMA-ring roles:
            #   sync   (SP HWDGE): x k-tiles + chain-stack gathers
            #   scalar (ACT HWDGE): operator stream + chain state copies
            #   vector (DVE): PSUM->SBUF output copies
            #   gpsimd (SWDGE): output stores
            xtiles = {}

            def issue_xk(t):
                if t in xtiles or t >= NT:
                    return
                xk = xpool.tile([P, BS], bf16, tag=f"x{t}", name=f"x{t}")
                nc.sync.dma_start(out=xk[:], in_=xT[P * t:P * t + P, :])
                xtiles[t] = xk

            # chain rhs tiles (state + full-column g0 stacks)
            ta = [apool.tile([P, BS], bf16, tag=f"a{i}", name=f"a{i}")
                  for i in range(_NSTEP)]
            tb = [bpool.tile([104, BS], bf16, tag=f"b{i}", name=f"b{i}")
                  for i in range(_NSTEP)]
            s_last = apool.tile([88, BS], bf16, tag="slast", name="slast")

            gathered = set()

            def issue_gathers(i):
                if i in gathered or i >= _NSTEP:
                    return
                gathered.add(i)
                for dst, dp, kt, sp, n in _gather_segs(i):
                    tile = ta[i] if dst == "a" else tb[i]
                    nc.sync.dma_start(out=tile[dp:dp + n, :],
                                      in_=xtiles[kt][sp:sp + n, :])

            wslot = {}
            wfetched = set()

            def fetch_w(gkey):
                if gkey in wfetched:
                    return
                wfetched.add(gkey)
                keys = dict(_FETCH)[gkey]
                cols = sum(_LAY[k][1] for k in keys)
                o0 = _LAY[keys[0]][0]
                wtile = wpool.tile([P, _WMAX], bf16, tag="w", name=f"w_{gkey}")
                nc.scalar.dma_start(out=wtile[:, :cols],
                                    in_=wt[:, o0:o0 + cols])
                for k in keys:
                    wslot[k] = (wtile, _LAY[k][0] - o0)

            def w_ap(key, kk):
                wtile, o = wslot[key]
                m = 88 if key[0].startswith("chain") else P
                return wtile[0:kk, o:o + m]

            # two independent batch-half pipelines: stream 0 = batch 0:512
            # (copies on scalar), stream 1 = batch 512:1024 (copies on DVE)
            def cp(m, dst, src):
                if m == 0:
                    nc.scalar.copy(dst, src)
                else:
                    nc.vector.tensor_copy(dst, src)

            def out_group(j, m):
                d = _JS[j]
                items = [("near", kt) for kt in d["kts"]]
                if d["cut"] is not None:
                    items.append(("far", None))
                ps = pso.tile([P, MW], f32, tag=f"o{m}", name=f"ps{j}_{m}")
                for it, (kind, kt) in enumerate(items):
                    first, last = it == 0, it == len(items) - 1
                    if kind == "near":
                        lhsT = w_ap(("near", j, kt), P)
                        rhs = xtiles[kt][:, m * MW:(m + 1) * MW]
                    else:
                        lhsT = w_ap(("far", j), 88)
                        st = ta[j - 1] if j - 1 < _NSTEP else s_last
                        rhs = st[0:88, m * MW:(m + 1) * MW]
                    nc.tensor.matmul(ps[:], lhsT=lhsT, rhs=rhs,
                                     start=first, stop=last)
                oc = opool.tile([P, MW], bf16, tag=f"o{m}", name=f"oc{j}_{m}")
                cp(m, oc[:], ps[:])
                nc.gpsimd.dma_start(
                    out=outT[P * j:P * j + P, m * MW:(m + 1) * MW], in_=oc[:])

            def chain_step(i, m):
                kk = _step_k(i)
                ka = min(kk, P)
                kb = kk - ka
                ps = pss.tile([88, MW], f32, tag=f"s{m}", name=f"pss{i}_{m}")
                items = [("chain_a", ka, ta[i])]
                if kb:
                    items.append(("chain_b", kb, tb[i]))
                for it, (wk, kdim, rt) in enumerate(items):
                    first, last = it == 0, it == len(items) - 1
                    nc.tensor.matmul(
                        ps[:], lhsT=w_ap((wk, i), kdim),
                        rhs=rt[0:kdim, m * MW:(m + 1) * MW],
                        start=first, stop=last)
                dst = ta[i + 1] if i + 1 < _NSTEP else s_last
                cp(m, dst[0:88, m * MW:(m + 1) * MW], ps[:])

            # ---------------- emission ----------------
            wqueue = [g for g, _ in _FETCH]  # consumption order
            wptr = 0

            def fetch_more(n):
                nonlocal wptr
                for _ in range(n):
                    if wptr < len(wqueue):
                        fetch_w(wqueue[wptr])
                        wptr += 1

            # PE warm-up: un-throttle HAM during the initial DMA window
            wu = xpool.tile([P, P], bf16, tag="warm", name="warm")
            nc.scalar.memzero(wu[:])
            pwu = pso.tile([P, MW], f32, tag="o0", name="pswarm")
            for _ in range(16):
                nc.tensor.matmul(pwu[:, 0:64], lhsT=wu[:], rhs=wu[:, 0:64],
                                 start=True, stop=True)
            issue_xk(0)
            issue_xk(1)
            issue_gathers(0)
            issue_xk(2)
            issue_gathers(1)
            issue_xk(3)
            issue_gathers(2)
            issue_xk(4)
            issue_gathers(3)
            fetch_more(len(wqueue))  # whole operator stream upfront
            # outputs run one chain-step behind: out(i+1) consumes s_{i-1},
            # whose PSUM->SBUF copy completed a full iteration earlier.
            for m in range(NM):
                out_group(0, m)
                out_group(1, m)
            chain_step(0, 0)
            chain_step(0, 1)
            for i in range(1, _NSTEP):
                issue_xk(i + 4)
                issue_gathers(i + 3)
                chain_step(i, 0)
                chain_step(i, 1)
                out_group(i + 1, 0)
                out_group(i + 1, 1)
            out_group(17, 0)
            out_group(17, 1)

    if not nc.is_finalized():
        nc.finalize()
    return nc


# -------------------------------------------------------------- driver ----

def kernel(inputs: np.ndarray, weights: np.ndarray) -> np.ndarray:
    import ml_dtypes
    from concourse.bass_utils import run_bass_kernel_spmd

    inputs = np.ascontiguousarray(inputs, dtype=np.float32)
    weights = np.ascontiguousarray(weights, dtype=np.float32)

    ops = _build_operators(weights)
    wt_packed = np.ascontiguousarray(_pack_ops(ops)).astype(ml_dtypes.bfloat16)

    # x-major per-sample flatten, then transpose so grid index leads
    xP = inputs.reshape(B, SIZE, SIZE).transpose(0, 2, 1).reshape(B, N)

    nc = _build_device_kernel()
    in_maps = [
        {
            "xT": np.ascontiguousarray(xP[c * BS:(c + 1) * BS].T)
            .astype(ml_dtypes.bfloat16),
            "wt": wt_packed,
        }
        for c in range(NCORES)
    ]
    trace = bool(int(os.environ.get("KERNEL_TRACE", "0")))
    res = run_bass_kernel_spmd(
        nc, in_maps, core_ids=list(range(NCORES)), trace=trace
    )
    if trace and res.exec_time_ns is not None:
        print(f"HW exec time: {res.exec_time_ns} ns")
        if res.instructions_and_trace is not None:
            print(f"trace: {res.instructions_and_trace[1]}")

    outP = np.concatenate(
        [res.results[c]["outT"].astype(np.float32).T for c in range(NCORES)],
        axis=0,
    )
    return np.ascontiguousarray(
        outP.reshape(B, SIZE, SIZE).transpose(0, 2, 1).reshape(B, N)
    )


# revision 37
# speedup vs baseline: 1.1745x; 1.1745x over previous
"""Trainium2 kernel for nn_Graph_41609643163904.

The reference op is a sequential per-cell scatter sweep over a 48x48 grid
(x outer, y inner): read center v, zero it, add v*W[y,x] to the 5x5
neighborhood.  Every step is linear in the grid, so the sweep is a fixed
linear operator M (2304x2304) of the weights; the baseline ran the full
block-banded M as 188 dense 128x128 matmul blocks.

v3 exploits the sweep's sequentially-semiseparable (SSS) structure: all
influence crossing grid-column x flows through the 88-dim interface
s(x) = [v(x); v(x-1)] (v = per-cell fired values, 44 per column).  In
x-major layout, each 128-row output tile j (covering grid columns
jc_lo..jc_hi) decomposes EXACTLY as

    out_j = sum_k  nearblk[j,k] @ g0_ktile[k]   (k over cols >= jc_lo: 2-3 tiles)
          + U_j @ s(jc_lo - 1)                  (everything to the left)

and the 16 needed states are propagated by a 16-step chain
    s_i = W_i.T @ [s_{i-1}; g0_interior(cols between cuts)]
with K<=128 per matmul.  Total: 47 near + 16 far + 33 chain lhsT blocks
= 95 logical matmuls vs the baseline's 188 dense blocks.  All operands
bf16 (tolerance is 2e-2; bf16 end-to-end lands ~5e-3), which also halves
DMA.  Data-parallel over the 8192-sample batch across 8 cores, no comm.

Device flow per core: x k-tiles + operator stream + interior-column
gathers (SBUF->SBUF) feed a single PE stream: near(j0), near(j1), then
[chain_i -> out(j=i+2)] interleaved so state copies hide under the near
matmuls of the following output tile.
"""

import os

import numpy as np

SIZE = 48
D = 2
KS = 5
N = SIZE * SIZE          # 2304
B = 8192
NCORES = 8
BS = B // NCORES         # 1024 samples per core
P = 128
NT = N // P              # 18 tiles of 128
MW = 512                 # matmul moving-dim (PSUM bank)
NM = BS // MW            # 2 halves


# ---------------------------------------------------------------- plan ----

def _plan():
    js = []
    for j in range(NT):
        r0 = P * j
        jc_lo, jc_hi = r0 // SIZE, (r0 + P - 1) // SIZE
        ncol_lo = jc_lo if j >= 2 else 0
        ncol_hi = min(jc_hi + 2, SIZE - 1)
        kt_lo = (SIZE * ncol_lo) // P
        kt_hi = (SIZE * (ncol_hi + 1) - 1) // P
        js.append(dict(j=j, jc_lo=jc_lo, jc_hi=jc_hi, ncol_lo=ncol_lo,
                       kts=list(range(kt_lo, kt_hi + 1)),
                       cut=jc_lo - 1 if j >= 2 else None))
    cuts = sorted({d["cut"] for d in js if d["cut"] is not None})
    return js, cuts


_JS, _XS = _plan()
_NSTEP = len(_XS)  # 16 chain steps (step 0 = init)


def _step_cols(i):
    """grid columns consumed by chain step i"""
    c0 = 2 if i == 0 else _XS[i - 1] + 1
    return c0, _XS[i]


def _step_ng(i):
    c0, c1 = _step_cols(i)
    return SIZE * (c1 - c0 + 1)        # FULL g0 columns (boundary rows get
                                       # zero operator coeffs)


def _step_pieces(i):
    """chain step i matmul pieces, accumulation order (state last).
    "sb" = [state(88); g0-overflow rows] packed in one rhs tile."""
    ng = _step_ng(i)
    ka = min(ng, P)
    if i == 0:
        out = [("ga", ka)]
        if ng > P:
            out.append(("gb", ng - P))
    else:
        out = [("ga", ka), ("sb", 88 + max(ng - P, 0))]
    return out


# ------------------------------------------------------- host operators ----

def _build_M_V(weights):
    """Composed operator M (N,N) and firing-value gradient rows V (1936,N),
    fp64, in the original y-major flattening."""
    M = np.eye(N, dtype=np.float64)
    V = np.zeros((44 * 44, N), dtype=np.float64)
    w = weights.astype(np.float64)
    for x in range(D, SIZE - D):
        for y in range(D, SIZE - D):
            c = y * SIZE + x
            wc = w[y, x]
            rc = M[c].copy()
            V[(x - D) * 44 + (y - D)] = rc
            for dy in range(-D, D + 1):
                r0 = c + dy * SIZE - D
                wrow = wc[dy + D]
                if dy == 0:
                    M[r0:r0 + D] += np.outer(wrow[:D], rc)
                    M[r0 + D + 1:r0 + KS] += np.outer(wrow[D + 1:], rc)
                else:
                    M[r0:r0 + KS] += np.outer(wrow, rc)
            M[c] = wc[D, D] * rc
    return M, V


def _xmajor_idx():
    n = np.arange(N)
    return (n % SIZE) * SIZE + n // SIZE


def _srows(X):
    return np.concatenate([(X - 2) * 44 + np.arange(44),
                           (X - 3) * 44 + np.arange(44)])


def _int_cols(c0, c1):
    return np.concatenate([SIZE * c + 2 + np.arange(44)
                           for c in range(c0, c1 + 1)])


def _build_operators(weights):
    M, V = _build_M_V(weights)
    idx = _xmajor_idx()
    Mx = M[np.ix_(idx, idx)]
    Vx = V[:, idx]
    ops = {}
    for d in _JS:
        j = d["j"]
        jr = slice(P * j, P * j + P)
        e_lo = SIZE * d["ncol_lo"]
        for kt in d["kts"]:
            blk = Mx[jr, P * kt:P * kt + P].copy()
            cols = np.arange(P * kt, P * kt + P)
            blk[:, cols < e_lo] = 0.0
            ops[("near", j, kt)] = blk.T        # lhsT (K=128, M=128)
        if d["cut"] is not None:
            X = d["cut"]
            sf = Vx[_srows(X), :SIZE * (X + 1)]
            F = Mx[jr, :SIZE * d["jc_lo"]]
            U, _, _, _ = np.linalg.lstsq(sf.T, F.T, rcond=None)
            ops[("far", j)] = U                 # lhsT (K=88, M=128)
    for i in range(_NSTEP):
        c0, c1 = _step_cols(i)
        X = _XS[i]
        Binj = Vx[_srows(X), SIZE * c0:SIZE * (c1 + 1)]  # full columns
        ka = min(_step_ng(i), P)
        if i == 0:
            ops[("chain_ga", 0)] = Binj.T[:ka]
            ops[("chain_gb", 0)] = Binj.T[ka:]
        else:
            Xp = _XS[i - 1]
            sf_p = Vx[_srows(Xp), :SIZE * (Xp + 1)]
            tgt = Vx[_srows(X), :SIZE * (Xp + 1)]
            T, _, _, _ = np.linalg.lstsq(sf_p.T, tgt.T, rcond=None)
            ops[("chain_ga", i)] = Binj.T[:ka]
            ops[("chain_sb", i)] = np.vstack([T, Binj.T[ka:]])
    return ops


# ----------------------------------------------------- operator packing ----

def _pack_layout():
    """Column ranges in the packed wt tensor, in PE consumption order."""
    off = 0
    lay = {}

    def put(key, cols):
        nonlocal off
        lay[key] = (off, cols)
        off += cols

    for kt in _JS[0]["kts"]:
        put(("near", 0, kt), P)
    for kt in _JS[1]["kts"]:
        put(("near", 1, kt), P)
    for i in range(_NSTEP):
        for pc, _k in _step_pieces(i):
            put((f"chain_{pc}", i), 88)
        j = i + 2
        for kt in _JS[j]["kts"]:
            put(("near", j, kt), P)
        put(("far", j), P)
    return lay, off


_LAY, _TOTC = _pack_layout()


def _pack_ops(ops):
    wt = np.zeros((P, _TOTC), dtype=np.float32)
    for d in _JS:
        j = d["j"]
        for kt in d["kts"]:
            o, c = _LAY[("near", j, kt)]
            wt[:, o:o + c] = ops[("near", j, kt)]
        if d["cut"] is not None:
            o, c = _LAY[("far", j)]
            wt[:88, o:o + P] = ops[("far", j)]
    for i in range(_NSTEP):
        for pc, k in _step_pieces(i):
            o, _ = _LAY[(f"chain_{pc}", i)]
            blk = ops[(f"chain_{pc}", i)]
            assert blk.shape[0] == k
            wt[:k, o:o + 88] = blk
    return wt


# fetch groups: (group key list of layout keys) in consumption order
def _fetch_groups():
    gs = []
    gs.append(("near0", [("near", 0, kt) for kt in _JS[0]["kts"]]))
    gs.append(("near1", [("near", 1, kt) for kt in _JS[1]["kts"]]))
    for i in range(_NSTEP):
        gs.append((f"ch{i}",
                   [(f"chain_{pc}", i) for pc, _k in _step_pieces(i)]))
        j = i + 2
        gs.append((f"out{j}",
                   [("near", j, kt) for kt in _JS[j]["kts"]] + [("far", j)]))
    return gs


_FETCH = _fetch_groups()
_WMAX = max(sum(_LAY[k][1] for k in keys) for _, keys in _FETCH)


# ------------------------------------------------------------- device ----

def _build_device_kernel():
    import concourse.mybir as mybir
    from concourse import bacc
    from concourse.tile import TileContext

    f32 = mybir.dt.float32
    bf16 = mybir.dt.bfloat16

    nc = bacc.Bacc()
    xT = nc.dram_tensor("xT", [N, BS], bf16, kind="ExternalInput")
    wt = nc.dram_tensor("wt", [P, _TOTC], bf16, kind="ExternalInput")
    # host-packed chain g0 stacks: step i at free block i
    xga = nc.dram_tensor("xga", [P, _NSTEP * BS], bf16, kind="ExternalInput")
    xgb = nc.dram_tensor("xgb", [16, _NSTEP * BS], bf16, kind="ExternalInput")
    outT = nc.dram_tensor("outT", [N, BS], bf16, kind="ExternalOutput")

    XCH = [4, 4, 4, 4, 2]   # x k-tiles merged into 5 SBUF tiles
    WSIZES = [3, 7, 6, 6, 6, 6]  # w stream chunk sizes (first small)

    with TileContext(nc) as tc:
        with (
            tc.tile_pool(name="xpool", bufs=1) as xpool,
            tc.tile_pool(name="spool", bufs=1) as spool,
            tc.tile_pool(name="wpool", bufs=1) as wpool,
            tc.tile_pool(name="opool", bufs=3) as opool,
            tc.tile_pool(name="pso", bufs=2, space="PSUM") as pso,
            tc.tile_pool(name="pss", bufs=2, space="PSUM") as pss,
        ):
            # Engine / ring roles:
            #   sync  (SP HWDGE): all input loads (x, xg, w)
            #   gpsimd (SWDGE):   output stores
            #   scalar/vector:    PSUM->SBUF copies, alternating
            xch = [xpool.tile([P, nk * BS], bf16, tag=f"x{ci}", name=f"x{ci}")
                   for ci, nk in enumerate(XCH)]
            kt0_of = [0, 4, 8, 12, 16]

            def issue_xk(kt):
                ci = min(kt // 4, 4)
                kk = kt - kt0_of[ci]
                nc.sync.dma_start(out=xch[ci][:, kk * BS:(kk + 1) * BS],
                                  in_=xT[P * kt:P * kt + P, :])

            def x_ap(kt, c0, c1):
                ci = min(kt // 4, 4)
                off = (kt - kt0_of[ci]) * BS
                return xch[ci][:, off + c0:off + c1]

            # chain g0 stacks: ga in 4 chunks (4 steps each), gb whole
            gach = [xpool.tile([P, 4 * BS], bf16, tag=f"ga{q}", name=f"ga{q}")
                    for q in range(4)]
            gb = xpool.tile([16, _NSTEP * BS], bf16, tag="gb", name="gb")

            # operator stream: merged chunk DMAs, consumption order
            wslot = {}
            wchunks = []
            nchunks = len(WSIZES)
            woff = [0]
            for s in WSIZES:
                woff.append(woff[-1] + s)
            assert woff[-1] == len(_FETCH)
            for ci in range(nchunks):
                grp = _FETCH[woff[ci]:woff[ci + 1]]
                keys = [k for _, ks in grp for k in ks]
                o0 = _LAY[keys[0]][0]
                cols = sum(_LAY[k][1] for k in keys)
                wtile = wpool.tile([P, 3584], bf16, tag=f"w{ci}",
                                   name=f"w{ci}")
                wchunks.append((wtile, o0, cols))
                for k in keys:
                    wslot[k] = (wtile, _LAY[k][0] - o0)

            def issue_w(ci):
                wtile, o0, cols = wchunks[ci]
                nc.sync.dma_start(out=wtile[:, :cols],
                                  in_=wt[:, o0:o0 + cols])

            def w_ap(key, kk):
                wtile, o = wslot[key]
                m = 88 if key[0].startswith("chain") else P
                return wtile[0:kk, o:o + m]

            # state tiles
            st = [spool.tile([104, BS], bf16, tag=f"s{i}", name=f"s{i}")
                  for i in range(_NSTEP)]

            def cp_state(dst, src):
                nc.scalar.copy(dst, src)

            def cp_out(dst, src):
                nc.vector.tensor_copy(dst, src)

            def out_group(j):
                d = _JS[j]
                items = [("near", kt) for kt in d["kts"]]
                if d["cut"] is not None:
                    items.append(("far", None))
                ps = pso.tile([P, BS], f32, tag="o", name=f"ps{j}")
                for it, (kind, kt) in enumerate(items):
                    first, last = it == 0, it == len(items) - 1
                    for m in range(NM):
                        if kind == "near":
                            lhsT = w_ap(("near", j, kt), P)
                            rhs = x_ap(kt, m * MW, (m + 1) * MW)
                        else:
                            lhsT = w_ap(("far", j), 88)
                            rhs = st[j - 2][0:88, m * MW:(m + 1) * MW]
                        nc.tensor.matmul(ps[:, m * MW:(m + 1) * MW],
                                         lhsT=lhsT, rhs=rhs,
                                         start=first, stop=last)
                oc = opool.tile([P, BS], bf16, tag="o", name=f"oc{j}")
                if j == 17:  # final tile: parallel half copies + stores
                    nc.vector.tensor_copy(oc[:, 0:MW], ps[:, 0:MW])
                    nc.scalar.copy(oc[:, MW:BS], ps[:, MW:BS])
                    nc.gpsimd.dma_start(out=outT[P * j:P * j + P, 0:MW],
                                        in_=oc[:, 0:MW])
                    nc.gpsimd.dma_start(out=outT[P * j:P * j + P, MW:BS],
                                        in_=oc[:, MW:BS])
                else:
                    cp_out(oc[:], ps[:])
                    nc.gpsimd.dma_start(out=outT[P * j:P * j + P, :],
                                        in_=oc[:])

            def chain_step(i):
                ps = pss.tile([88, BS], f32, tag="s", name=f"pss{i}")
                pieces = _step_pieces(i)
                for it, (pc, kdim) in enumerate(pieces):
                    first, last = it == 0, it == len(pieces) - 1
                    if pc == "ga":
                        rt, base = gach[i // 4], (i % 4) * BS
                    elif pc == "gb":
                        rt, base = gb, i * BS
                    else:
                        rt, base = st[i - 1], 0
                    for m in range(NM):
                        nc.tensor.matmul(
                            ps[:, m * MW:(m + 1) * MW],
                            lhsT=w_ap((f"chain_{pc}", i), kdim),
                            rhs=rt[0:kdim, base + m * MW:base + (m + 1) * MW],
                            start=first, stop=last)
                cp_state(st[i][0:88, :], ps[:])

            # ---------------- emission ----------------
            # PE warm-up: un-throttle HAM during the initial DMA window
            wu = spool.tile([P, P], bf16, tag="warm", name="warm")
            nc.scalar.memzero(wu[:])
            pwu = pso.tile([P, BS], f32, tag="o", name="pswarm")
            for _ in range(12):
                nc.tensor.matmul(pwu[:, 0:64], lhsT=wu[:], rhs=wu[:, 0:64],
                                 start=True, stop=True)

            issue_xk(0)
            issue_xk(1)
            issue_w(0)
            nc.sync.dma_start(out=gach[0][:],
                              in_=xga[:, 0:4 * BS])
            nc.sync.dma_start(out=gb[:, 0:BS], in_=xgb[:, 0:BS])
            issue_xk(2)
            issue_xk(3)
            issue_w(1)
            nc.sync.dma_start(out=gach[1][:],
                              in_=xga[:, 4 * BS:8 * BS])

            def issue_ov(i):
                ov = max(_step_ng(i) - P, 0)
                if ov:
                    nc.sync.dma_start(
                        out=st[i - 1][88:88 + ov, :],
                        in_=xgb[0:ov, i * BS:(i + 1) * BS])

            for i in range(1, 6):
                issue_ov(i)
            for t in range(4, NT):
                issue_xk(t)
                if t in (5, 8, 11, 14):
                    ci = 2 + (t - 5) // 3
                    if ci < nchunks:
                        issue_w(ci)
                if t == 5:
                    nc.sync.dma_start(out=gach[2][:],
                                      in_=xga[:, 8 * BS:12 * BS])
                    for i in range(6, 11):
                        issue_ov(i)
                if t == 8:
                    nc.sync.dma_start(out=gach[3][:],
                                      in_=xga[:, 12 * BS:16 * BS])
                    for i in range(11, _NSTEP):
                        issue_ov(i)

            out_group(0)
            out_group(1)
            chain_step(0)
            for i in range(1, _NSTEP):
                chain_step(i)
                out_group(i + 1)
            out_group(17)

    if not nc.is_finalized():
        nc.finalize()
    return nc


# -------------------------------------------------------------- driver ----

def kernel(inputs: np.ndarray, weights: np.ndarray) -> np.ndarray:
    import ml_dtypes
    from concourse.bass_utils import run_bass_kernel_spmd

    inputs = np.ascontiguousarray(inputs, dtype=np.float32)
    weights = np.ascontiguousarray(weights, dtype=np.float32)

    ops = _build_operators(weights)
    wt_packed = np.ascontiguousarray(_pack_ops(ops)).astype(ml_dtypes.bfloat16)

    # x-major per-sample flatten, then transpose so grid index leads
    xP = inputs.reshape(B, SIZE, SIZE).transpose(0, 2, 1).reshape(B, N)

    nc = _build_device_kernel()
    in_maps = []
    for c in range(NCORES):
        xc = np.ascontiguousarray(xP[c * BS:(c + 1) * BS].T)  # (N, BS) fp32
        xga = np.zeros((P, _NSTEP * BS), dtype=np.float32)
        xgb = np.zeros((16, _NSTEP * BS), dtype=np.float32)
        for i in range(_NSTEP):
            c0, _c1 = _step_cols(i)
            ng = _step_ng(i)
            r0 = SIZE * c0
            ka = min(ng, P)
            xga[:ka, i * BS:(i + 1) * BS] = xc[r0:r0 + ka]
            if ng > P:
                xgb[:ng - P, i * BS:(i + 1) * BS] = xc[r0 + P:r0 + ng]
        in_maps.append({
            "xT": xc.astype(ml_dtypes.bfloat16),
            "wt": wt_packed,
            "xga": xga.astype(ml_dtypes.bfloat16),
            "xgb": xgb.astype(ml_dtypes.bfloat16),
        })
    trace = bool(int(os.environ.get("KERNEL_TRACE", "0")))
    res = run_bass_kernel_spmd(
        nc, in_maps, core_ids=list(range(NCORES)), trace=trace
    )
    if trace and res.exec_time_ns is not None:
        print(f"HW exec time: {res.exec_time_ns} ns")
        if res.instructions_and_trace is not None:
            print(f"trace: {res.instructions_and_trace[1]}")

    outP = np.concatenate(
        [res.results[c]["outT"].astype(np.float32).T for c in range(NCORES)],
        axis=0,
    )
    return np.ascontiguousarray(
        outP.reshape(B, SIZE, SIZE).transpose(0, 2, 1).reshape(B, N)
    )


# revision 38
# speedup vs baseline: 1.2107x; 1.0308x over previous
"""Trainium2 kernel for nn_Graph_41609643163904.

The reference op is a sequential per-cell scatter sweep over a 48x48 grid
(x outer, y inner): read center v, zero it, add v*W[y,x] to the 5x5
neighborhood.  Every step is linear in the grid, so the sweep is a fixed
linear operator M (2304x2304) of the weights; the baseline ran the full
block-banded M as 188 dense 128x128 matmul blocks (fp32r).

This version exploits the sweep's sequentially-semiseparable structure:
all influence crossing grid-column x flows through the 88-dim interface
s(x) = [v(x); v(x-1)] (v = per-cell fired values, 44 per column).  In
x-major layout, each 128-row output tile j (covering grid columns
jc_lo..jc_hi) decomposes EXACTLY as

    out_j = sum_k  nearblk[j,k] @ g0_ktile[k]   (cols >= jc_lo: 2-3 k-tiles)
          + U_j @ s(jc_lo - 1)                  (everything to the left)

with the 16 needed states propagated by a 16-step chain whose rhs is
[host-packed g0 column stack (<=128) | state(88)+overflow(<=16)], two
K<=128 matmuls per step.  Total 206 N=512 matmuls vs the baseline's 376.
All operands bf16 (tolerance 2e-2; end-to-end lands 5.4e-3), halving DMA.
Data-parallel over the 8192-sample batch across 8 cores, no comm.

Hardware lessons baked into the schedule (each worth ~10-20us):
  * every dma_start costs ~0.7us of its sequencer: loads are merged into
    a few big DMAs (5 x-chunks as 18 contiguous-source pieces, 6 operator
    chunks, 4 chain-stack chunks) issued on the idle SP ring
  * PSUM->SBUF copies cost ~1.2us nearly flat in size: one big copy per
    PSUM tile, out-copies on DVE, state copies on ACT (disjoint queues)
  * outputs run one chain-step behind state production so every matmul's
    semaphores are pre-satisfied; chain accumulates g0 first, state last
  * 12 dummy matmuls at t=0 hold off the HAM 1.2GHz idle-throttle while
    the first loads are in flight
"""

import os

import numpy as np

SIZE = 48
D = 2
KS = 5
N = SIZE * SIZE          # 2304
B = 8192
NCORES = 8
BS = B // NCORES         # 1024 samples per core
P = 128
NT = N // P              # 18 tiles of 128
MW = 512                 # matmul moving-dim (PSUM bank)
NM = BS // MW            # 2 halves


# ---------------------------------------------------------------- plan ----

def _plan():
    js = []
    for j in range(NT):
        r0 = P * j
        jc_lo, jc_hi = r0 // SIZE, (r0 + P - 1) // SIZE
        ncol_lo = jc_lo if j >= 2 else 0
        ncol_hi = min(jc_hi + 2, SIZE - 1)
        kt_lo = (SIZE * ncol_lo) // P
        kt_hi = (SIZE * (ncol_hi + 1) - 1) // P
        js.append(dict(j=j, jc_lo=jc_lo, jc_hi=jc_hi, ncol_lo=ncol_lo,
                       kts=list(range(kt_lo, kt_hi + 1)),
                       cut=jc_lo - 1 if j >= 2 else None))
    cuts = sorted({d["cut"] for d in js if d["cut"] is not None})
    return js, cuts


_JS, _XS = _plan()
_NSTEP = len(_XS)  # 16 chain steps (step 0 = init)


def _step_cols(i):
    """grid columns consumed by chain step i"""
    c0 = 2 if i == 0 else _XS[i - 1] + 1
    return c0, _XS[i]


def _step_ng(i):
    c0, c1 = _step_cols(i)
    return SIZE * (c1 - c0 + 1)        # FULL g0 columns (boundary rows get
                                       # zero operator coeffs)


def _step_pieces(i):
    """chain step i matmul pieces, accumulation order (state last).
    "sb" = [state(88); g0-overflow rows] packed in one rhs tile."""
    ng = _step_ng(i)
    ka = min(ng, P)
    if i == 0:
        out = [("ga", ka)]
        if ng > P:
            out.append(("gb", ng - P))
    else:
        out = [("ga", ka), ("sb", 88 + max(ng - P, 0))]
    return out


# ------------------------------------------------------- host operators ----

def _build_M_V(weights):
    """Composed operator M (N,N) and firing-value gradient rows V (1936,N),
    fp64, in the original y-major flattening."""
    M = np.eye(N, dtype=np.float64)
    V = np.zeros((44 * 44, N), dtype=np.float64)
    w = weights.astype(np.float64)
    for x in range(D, SIZE - D):
        for y in range(D, SIZE - D):
            c = y * SIZE + x
            wc = w[y, x]
            rc = M[c].copy()
            V[(x - D) * 44 + (y - D)] = rc
            for dy in range(-D, D + 1):
                r0 = c + dy * SIZE - D
                wrow = wc[dy + D]
                if dy == 0:
                    M[r0:r0 + D] += np.outer(wrow[:D], rc)
                    M[r0 + D + 1:r0 + KS] += np.outer(wrow[D + 1:], rc)
                else:
                    M[r0:r0 + KS] += np.outer(wrow, rc)
            M[c] = wc[D, D] * rc
    return M, V


def _xmajor_idx():
    n = np.arange(N)
    return (n % SIZE) * SIZE + n // SIZE


def _srows(X):
    return np.concatenate([(X - 2) * 44 + np.arange(44),
                           (X - 3) * 44 + np.arange(44)])


def _build_operators(weights):
    M, V = _build_M_V(weights)
    idx = _xmajor_idx()
    Mx = M[np.ix_(idx, idx)]
    Vx = V[:, idx]
    ops = {}
    for d in _JS:
        j = d["j"]
        jr = slice(P * j, P * j + P)
        e_lo = SIZE * d["ncol_lo"]
        for kt in d["kts"]:
            blk = Mx[jr, P * kt:P * kt + P].copy()
            cols = np.arange(P * kt, P * kt + P)
            blk[:, cols < e_lo] = 0.0
            ops[("near", j, kt)] = blk.T        # lhsT (K=128, M=128)
        if d["cut"] is not None:
            X = d["cut"]
            sf = Vx[_srows(X), :SIZE * (X + 1)]
            F = Mx[jr, :SIZE * d["jc_lo"]]
            U, _, _, _ = np.linalg.lstsq(sf.T, F.T, rcond=None)
            ops[("far", j)] = U                 # lhsT (K=88, M=128)
    for i in range(_NSTEP):
        c0, c1 = _step_cols(i)
        X = _XS[i]
        Binj = Vx[_srows(X), SIZE * c0:SIZE * (c1 + 1)]  # full columns
        ka = min(_step_ng(i), P)
        if i == 0:
            ops[("chain_ga", 0)] = Binj.T[:ka]
            ops[("chain_gb", 0)] = Binj.T[ka:]
        else:
            Xp = _XS[i - 1]
            sf_p = Vx[_srows(Xp), :SIZE * (Xp + 1)]
            tgt = Vx[_srows(X), :SIZE * (Xp + 1)]
            T, _, _, _ = np.linalg.lstsq(sf_p.T, tgt.T, rcond=None)
            ops[("chain_ga", i)] = Binj.T[:ka]
            ops[("chain_sb", i)] = np.vstack([T, Binj.T[ka:]])
    return ops


# ----------------------------------------------------- operator packing ----

def _pack_layout():
    """Column ranges in the packed wt tensor, in PE consumption order."""
    off = 0
    lay = {}

    def put(key, cols):
        nonlocal off
        lay[key] = (off, cols)
        off += cols

    for kt in _JS[0]["kts"]:
        put(("near", 0, kt), P)
    for kt in _JS[1]["kts"]:
        put(("near", 1, kt), P)
    for i in range(_NSTEP):
        for pc, _k in _step_pieces(i):
            put((f"chain_{pc}", i), 88)
        j = i + 2
        for kt in _JS[j]["kts"]:
            put(("near", j, kt), P)
        put(("far", j), P)
    return lay, off


_LAY, _TOTC = _pack_layout()


def _pack_ops(ops):
    wt = np.zeros((P, _TOTC), dtype=np.float32)
    for d in _JS:
        j = d["j"]
        for kt in d["kts"]:
            o, c = _LAY[("near", j, kt)]
            wt[:, o:o + c] = ops[("near", j, kt)]
        if d["cut"] is not None:
            o, c = _LAY[("far", j)]
            wt[:88, o:o + P] = ops[("far", j)]
    for i in range(_NSTEP):
        for pc, k in _step_pieces(i):
            o, _ = _LAY[(f"chain_{pc}", i)]
            blk = ops[(f"chain_{pc}", i)]
            assert blk.shape[0] == k
            wt[:k, o:o + 88] = blk
    return wt


# fetch groups: (group key list of layout keys) in consumption order
def _fetch_groups():
    gs = []
    gs.append(("near0", [("near", 0, kt) for kt in _JS[0]["kts"]]))
    gs.append(("near1", [("near", 1, kt) for kt in _JS[1]["kts"]]))
    for i in range(_NSTEP):
        gs.append((f"ch{i}",
                   [(f"chain_{pc}", i) for pc, _k in _step_pieces(i)]))
        j = i + 2
        gs.append((f"out{j}",
                   [("near", j, kt) for kt in _JS[j]["kts"]] + [("far", j)]))
    return gs


_FETCH = _fetch_groups()
_WMAX = max(sum(_LAY[k][1] for k in keys) for _, keys in _FETCH)


# ------------------------------------------------------------- device ----

def _build_device_kernel():
    import concourse.mybir as mybir
    from concourse import bacc
    from concourse.tile import TileContext

    f32 = mybir.dt.float32
    bf16 = mybir.dt.bfloat16

    nc = bacc.Bacc()
    xT = nc.dram_tensor("xT", [N, BS], bf16, kind="ExternalInput")
    wt = nc.dram_tensor("wt", [P, _TOTC], bf16, kind="ExternalInput")
    # host-packed chain g0 stacks: step i at free block i
    xga = nc.dram_tensor("xga", [P, _NSTEP * BS], bf16, kind="ExternalInput")
    xgb = nc.dram_tensor("xgb", [16, _NSTEP * BS], bf16, kind="ExternalInput")
    outT = nc.dram_tensor("outT", [N, BS], bf16, kind="ExternalOutput")

    XCH = [4, 4, 4, 4, 2]   # x k-tiles merged into 5 SBUF tiles
    WSIZES = [3, 7, 6, 6, 6, 6]  # w stream chunk sizes (first small)

    with TileContext(nc) as tc:
        with (
            tc.tile_pool(name="xpool", bufs=1) as xpool,
            tc.tile_pool(name="spool", bufs=1) as spool,
            tc.tile_pool(name="wpool", bufs=1) as wpool,
            tc.tile_pool(name="opool", bufs=3) as opool,
            tc.tile_pool(name="pso", bufs=2, space="PSUM") as pso,
            tc.tile_pool(name="pss", bufs=2, space="PSUM") as pss,
        ):
            # Engine / ring roles:
            #   sync  (SP HWDGE): all input loads (x, xg, w)
            #   gpsimd (SWDGE):   output stores
            #   scalar/vector:    PSUM->SBUF copies, alternating
            xch = [xpool.tile([P, nk * BS], bf16, tag=f"x{ci}", name=f"x{ci}")
                   for ci, nk in enumerate(XCH)]
            kt0_of = [0, 4, 8, 12, 16]

            def issue_xk(kt):
                ci = min(kt // 4, 4)
                kk = kt - kt0_of[ci]
                nc.sync.dma_start(out=xch[ci][:, kk * BS:(kk + 1) * BS],
                                  in_=xT[P * kt:P * kt + P, :])

            def x_ap(kt, c0, c1):
                ci = min(kt // 4, 4)
                off = (kt - kt0_of[ci]) * BS
                return xch[ci][:, off + c0:off + c1]

            # chain g0 stacks: ga in 4 chunks (4 steps each), gb whole
            gach = [xpool.tile([P, 4 * BS], bf16, tag=f"ga{q}", name=f"ga{q}")
                    for q in range(4)]
            gb = xpool.tile([16, _NSTEP * BS], bf16, tag="gb", name="gb")

            # operator stream: merged chunk DMAs, consumption order
            wslot = {}
            wchunks = []
            nchunks = len(WSIZES)
            woff = [0]
            for s in WSIZES:
                woff.append(woff[-1] + s)
            assert woff[-1] == len(_FETCH)
            for ci in range(nchunks):
                grp = _FETCH[woff[ci]:woff[ci + 1]]
                keys = [k for _, ks in grp for k in ks]
                o0 = _LAY[keys[0]][0]
                cols = sum(_LAY[k][1] for k in keys)
                wtile = wpool.tile([P, 3584], bf16, tag=f"w{ci}",
                                   name=f"w{ci}")
                wchunks.append((wtile, o0, cols))
                for k in keys:
                    wslot[k] = (wtile, _LAY[k][0] - o0)

            def issue_w(ci):
                wtile, o0, cols = wchunks[ci]
                nc.sync.dma_start(out=wtile[:, :cols],
                                  in_=wt[:, o0:o0 + cols])

            def w_ap(key, kk):
                wtile, o = wslot[key]
                m = 88 if key[0].startswith("chain") else P
                return wtile[0:kk, o:o + m]

            # state tiles
            st = [spool.tile([104, BS], bf16, tag=f"s{i}", name=f"s{i}")
                  for i in range(_NSTEP)]

            def cp_state(dst, src):
                nc.scalar.copy(dst, src)

            def cp_out(dst, src):
                nc.vector.tensor_copy(dst, src)

            def out_group(j):
                d = _JS[j]
                items = [("near", kt) for kt in d["kts"]]
                if d["cut"] is not None:
                    items.append(("far", None))
                ps = pso.tile([P, BS], f32, tag="o", name=f"ps{j}")
                for it, (kind, kt) in enumerate(items):
                    first, last = it == 0, it == len(items) - 1
                    for m in range(NM):
                        if kind == "near":
                            lhsT = w_ap(("near", j, kt), P)
                            rhs = x_ap(kt, m * MW, (m + 1) * MW)
                        else:
                            lhsT = w_ap(("far", j), 88)
                            rhs = st[j - 2][0:88, m * MW:(m + 1) * MW]
                        nc.tensor.matmul(ps[:, m * MW:(m + 1) * MW],
                                         lhsT=lhsT, rhs=rhs,
                                         start=first, stop=last)
                oc = opool.tile([P, BS], bf16, tag="o", name=f"oc{j}")
                if j == 17:  # final tile: parallel half copies + stores
                    nc.vector.tensor_copy(oc[:, 0:MW], ps[:, 0:MW])
                    nc.scalar.copy(oc[:, MW:BS], ps[:, MW:BS])
                    nc.gpsimd.dma_start(out=outT[P * j:P * j + P, 0:MW],
                                        in_=oc[:, 0:MW])
                    nc.gpsimd.dma_start(out=outT[P * j:P * j + P, MW:BS],
                                        in_=oc[:, MW:BS])
                else:
                    cp_out(oc[:], ps[:])
                    nc.gpsimd.dma_start(out=outT[P * j:P * j + P, :],
                                        in_=oc[:])

            def chain_step(i):
                ps = pss.tile([88, BS], f32, tag="s", name=f"pss{i}")
                pieces = _step_pieces(i)
                for it, (pc, kdim) in enumerate(pieces):
                    first, last = it == 0, it == len(pieces) - 1
                    if pc == "ga":
                        rt, base = gach[i // 4], (i % 4) * BS
                    elif pc == "gb":
                        rt, base = gb, i * BS
                    else:
                        rt, base = st[i - 1], 0
                    for m in range(NM):
                        nc.tensor.matmul(
                            ps[:, m * MW:(m + 1) * MW],
                            lhsT=w_ap((f"chain_{pc}", i), kdim),
                            rhs=rt[0:kdim, base + m * MW:base + (m + 1) * MW],
                            start=first, stop=last)
                cp_state(st[i][0:88, :], ps[:])

            # ---------------- emission ----------------
            # PE warm-up: un-throttle HAM during the initial DMA window
            wu = spool.tile([P, P], bf16, tag="warm", name="warm")
            nc.scalar.memzero(wu[:])
            pwu = pso.tile([P, BS], f32, tag="o", name="pswarm")
            for _ in range(12):
                nc.tensor.matmul(pwu[:, 0:64], lhsT=wu[:], rhs=wu[:, 0:64],
                                 start=True, stop=True)

            issue_xk(0)
            issue_xk(1)
            issue_w(0)
            nc.sync.dma_start(out=gach[0][:],
                              in_=xga[:, 0:4 * BS])
            nc.sync.dma_start(out=gb[:, 0:BS], in_=xgb[:, 0:BS])
            issue_xk(2)
            issue_xk(3)
            issue_w(1)
            nc.sync.dma_start(out=gach[1][:],
                              in_=xga[:, 4 * BS:8 * BS])

            def issue_ov(i):
                ov = max(_step_ng(i) - P, 0)
                if ov:
                    nc.sync.dma_start(
                        out=st[i - 1][88:88 + ov, :],
                        in_=xgb[0:ov, i * BS:(i + 1) * BS])

            for i in range(1, 6):
                issue_ov(i)
            for t in range(4, NT):
                issue_xk(t)
                if t in (5, 8, 11, 14):
                    ci = 2 + (t - 5) // 3
                    if ci < nchunks:
                        issue_w(ci)
                if t == 5:
                    nc.sync.dma_start(out=gach[2][:],
                                      in_=xga[:, 8 * BS:12 * BS])
                    for i in range(6, 11):
                        issue_ov(i)
                if t == 8:
                    nc.sync.dma_start(out=gach[3][:],
                                      in_=xga[:, 12 * BS:16 * BS])
                    for i in range(11, _NSTEP):
                        issue_ov(i)

            out_group(0)
            out_group(1)
            chain_step(0)
            for i in range(1, _NSTEP):
                chain_step(i)
                out_group(i + 1)
            out_group(17)

    if not nc.is_finalized():
        nc.finalize()
    return nc


# -------------------------------------------------------------- driver ----

def kernel(inputs: np.ndarray, weights: np.ndarray) -> np.ndarray:
    import ml_dtypes
    from concourse.bass_utils import run_bass_kernel_spmd

    inputs = np.ascontiguousarray(inputs, dtype=np.float32)
    weights = np.ascontiguousarray(weights, dtype=np.float32)

    ops = _build_operators(weights)
    wt_packed = np.ascontiguousarray(_pack_ops(ops)).astype(ml_dtypes.bfloat16)

    # x-major per-sample flatten, then transpose so grid index leads
    xP = inputs.reshape(B, SIZE, SIZE).transpose(0, 2, 1).reshape(B, N)

    nc = _build_device_kernel()
    in_maps = []
    for c in range(NCORES):
        xc = np.ascontiguousarray(xP[c * BS:(c + 1) * BS].T)  # (N, BS) fp32
        xga = np.zeros((P, _NSTEP * BS), dtype=np.float32)
        xgb = np.zeros((16, _NSTEP * BS), dtype=np.float32)
        for i in range(_NSTEP):
            c0, _c1 = _step_cols(i)
            ng = _step_ng(i)
            r0 = SIZE * c0
            ka = min(ng, P)
            xga[:ka, i * BS:(i + 1) * BS] = xc[r0:r0 + ka]
            if ng > P:
                xgb[:ng - P, i * BS:(i + 1) * BS] = xc[r0 + P:r0 + ng]
        in_maps.append({
            "xT": xc.astype(ml_dtypes.bfloat16),
            "wt": wt_packed,
            "xga": xga.astype(ml_dtypes.bfloat16),
            "xgb": xgb.astype(ml_dtypes.bfloat16),
        })
    trace = bool(int(os.environ.get("KERNEL_TRACE", "0")))
    res = run_bass_kernel_spmd(
        nc, in_maps, core_ids=list(range(NCORES)), trace=trace
    )
    if trace and res.exec_time_ns is not None:
        print(f"HW exec time: {res.exec_time_ns} ns")
        if res.instructions_and_trace is not None:
            print(f"trace: {res.instructions_and_trace[1]}")

    outP = np.concatenate(
        [res.results[c]["outT"].astype(np.float32).T for c in range(NCORES)],
        axis=0,
    )
    return np.ascontiguousarray(
        outP.reshape(B, SIZE, SIZE).transpose(0, 2, 1).reshape(B, N)
    )


# revision 39
# speedup vs baseline: 1.2142x; 1.0029x over previous
"""Trainium2 kernel for nn_Graph_41609643163904.

The reference op is a sequential per-cell scatter sweep over a 48x48 grid
(x outer, y inner): read center v, zero it, add v*W[y,x] to the 5x5
neighborhood.  Every step is linear in the grid, so the sweep is a fixed
linear operator M (2304x2304) of the weights; the baseline ran the full
block-banded M as 188 dense 128x128 matmul blocks (fp32r).

This version exploits the sweep's sequentially-semiseparable structure:
all influence crossing grid-column x flows through the 88-dim interface
s(x) = [v(x); v(x-1)] (v = per-cell fired values, 44 per column).  In
x-major layout, each 128-row output tile j (covering grid columns
jc_lo..jc_hi) decomposes EXACTLY as

    out_j = sum_k  nearblk[j,k] @ g0_ktile[k]   (cols >= jc_lo: 2-3 k-tiles)
          + U_j @ s(jc_lo - 1)                  (everything to the left)

with the 16 needed states propagated by a 16-step chain whose rhs is
[host-packed g0 column stack (<=128) | state(88)+overflow(<=16)], two
K<=128 matmuls per step.  Total 206 N=512 matmuls vs the baseline's 376.
All operands bf16 (tolerance 2e-2; end-to-end lands 5.4e-3), halving DMA.
Data-parallel over the 8192-sample batch across 8 cores, no comm.

Hardware lessons baked into the schedule (each worth ~10-20us):
  * every dma_start costs ~0.7us of its sequencer: loads are merged into
    a few big DMAs (5 x-chunks as 18 contiguous-source pieces, 6 operator
    chunks, 4 chain-stack chunks) issued on the idle SP ring
  * PSUM->SBUF copies cost ~1.2us nearly flat in size: one big copy per
    PSUM tile, out-copies on DVE, state copies on ACT (disjoint queues)
  * outputs run one chain-step behind state production so every matmul's
    semaphores are pre-satisfied; chain accumulates g0 first, state last
  * 12 dummy matmuls at t=0 hold off the HAM 1.2GHz idle-throttle while
    the first loads are in flight
"""

import os

import numpy as np

SIZE = 48
D = 2
KS = 5
N = SIZE * SIZE          # 2304
B = 8192
NCORES = 8
BS = B // NCORES         # 1024 samples per core
P = 128
NT = N // P              # 18 tiles of 128
MW = 512                 # matmul moving-dim (PSUM bank)
NM = BS // MW            # 2 halves


# ---------------------------------------------------------------- plan ----

def _plan():
    js = []
    for j in range(NT):
        r0 = P * j
        jc_lo, jc_hi = r0 // SIZE, (r0 + P - 1) // SIZE
        ncol_lo = jc_lo if j >= 2 else 0
        ncol_hi = min(jc_hi + 2, SIZE - 1)
        kt_lo = (SIZE * ncol_lo) // P
        kt_hi = (SIZE * (ncol_hi + 1) - 1) // P
        js.append(dict(j=j, jc_lo=jc_lo, jc_hi=jc_hi, ncol_lo=ncol_lo,
                       kts=list(range(kt_lo, kt_hi + 1)),
                       cut=jc_lo - 1 if j >= 2 else None))
    cuts = sorted({d["cut"] for d in js if d["cut"] is not None})
    return js, cuts


_JS, _XS = _plan()
_NSTEP = len(_XS)  # 16 chain steps (step 0 = init)


def _step_cols(i):
    """grid columns consumed by chain step i"""
    c0 = 2 if i == 0 else _XS[i - 1] + 1
    return c0, _XS[i]


def _step_ng(i):
    c0, c1 = _step_cols(i)
    return SIZE * (c1 - c0 + 1)        # FULL g0 columns (boundary rows get
                                       # zero operator coeffs)


def _step_pieces(i):
    """chain step i matmul pieces, accumulation order (state last).
    "sb" = [state(88); g0-overflow rows] packed in one rhs tile."""
    ng = _step_ng(i)
    ka = min(ng, P)
    if i == 0:
        out = [("ga", ka)]
        if ng > P:
            out.append(("gb", ng - P))
    else:
        out = [("ga", ka), ("sb", 88 + max(ng - P, 0))]
    return out


# ------------------------------------------------------- host operators ----

def _build_M_V(weights):
    """Composed operator M (N,N) and firing-value gradient rows V (1936,N),
    fp64, in the original y-major flattening."""
    M = np.eye(N, dtype=np.float64)
    V = np.zeros((44 * 44, N), dtype=np.float64)
    w = weights.astype(np.float64)
    for x in range(D, SIZE - D):
        for y in range(D, SIZE - D):
            c = y * SIZE + x
            wc = w[y, x]
            rc = M[c].copy()
            V[(x - D) * 44 + (y - D)] = rc
            for dy in range(-D, D + 1):
                r0 = c + dy * SIZE - D
                wrow = wc[dy + D]
                if dy == 0:
                    M[r0:r0 + D] += np.outer(wrow[:D], rc)
                    M[r0 + D + 1:r0 + KS] += np.outer(wrow[D + 1:], rc)
                else:
                    M[r0:r0 + KS] += np.outer(wrow, rc)
            M[c] = wc[D, D] * rc
    return M, V


def _xmajor_idx():
    n = np.arange(N)
    return (n % SIZE) * SIZE + n // SIZE


def _srows(X):
    return np.concatenate([(X - 2) * 44 + np.arange(44),
                           (X - 3) * 44 + np.arange(44)])


def _build_operators(weights):
    M, V = _build_M_V(weights)
    idx = _xmajor_idx()
    Mx = M[np.ix_(idx, idx)]
    Vx = V[:, idx]
    ops = {}
    for d in _JS:
        j = d["j"]
        jr = slice(P * j, P * j + P)
        e_lo = SIZE * d["ncol_lo"]
        for kt in d["kts"]:
            blk = Mx[jr, P * kt:P * kt + P].copy()
            cols = np.arange(P * kt, P * kt + P)
            blk[:, cols < e_lo] = 0.0
            ops[("near", j, kt)] = blk.T        # lhsT (K=128, M=128)
        if d["cut"] is not None:
            X = d["cut"]
            sf = Vx[_srows(X), :SIZE * (X + 1)]
            F = Mx[jr, :SIZE * d["jc_lo"]]
            U, _, _, _ = np.linalg.lstsq(sf.T, F.T, rcond=None)
            ops[("far", j)] = U                 # lhsT (K=88, M=128)
    for i in range(_NSTEP):
        c0, c1 = _step_cols(i)
        X = _XS[i]
        Binj = Vx[_srows(X), SIZE * c0:SIZE * (c1 + 1)]  # full columns
        ka = min(_step_ng(i), P)
        if i == 0:
            ops[("chain_ga", 0)] = Binj.T[:ka]
            ops[("chain_gb", 0)] = Binj.T[ka:]
        else:
            Xp = _XS[i - 1]
            sf_p = Vx[_srows(Xp), :SIZE * (Xp + 1)]
            tgt = Vx[_srows(X), :SIZE * (Xp + 1)]
            T, _, _, _ = np.linalg.lstsq(sf_p.T, tgt.T, rcond=None)
            ops[("chain_ga", i)] = Binj.T[:ka]
            ops[("chain_sb", i)] = np.vstack([T, Binj.T[ka:]])
    return ops


# ----------------------------------------------------- operator packing ----

def _pack_layout():
    """Column ranges in the packed wt tensor, in PE consumption order."""
    off = 0
    lay = {}

    def put(key, cols):
        nonlocal off
        lay[key] = (off, cols)
        off += cols

    for kt in _JS[0]["kts"]:
        put(("near", 0, kt), P)
    for kt in _JS[1]["kts"]:
        put(("near", 1, kt), P)
    for i in range(_NSTEP):
        for pc, _k in _step_pieces(i):
            put((f"chain_{pc}", i), 88)
        j = i + 2
        for kt in _JS[j]["kts"]:
            put(("near", j, kt), P)
        put(("far", j), P)
    return lay, off


_LAY, _TOTC = _pack_layout()


def _pack_ops(ops):
    wt = np.zeros((P, _TOTC), dtype=np.float32)
    for d in _JS:
        j = d["j"]
        for kt in d["kts"]:
            o, c = _LAY[("near", j, kt)]
            wt[:, o:o + c] = ops[("near", j, kt)]
        if d["cut"] is not None:
            o, c = _LAY[("far", j)]
            wt[:88, o:o + P] = ops[("far", j)]
    for i in range(_NSTEP):
        for pc, k in _step_pieces(i):
            o, _ = _LAY[(f"chain_{pc}", i)]
            blk = ops[(f"chain_{pc}", i)]
            assert blk.shape[0] == k
            wt[:k, o:o + 88] = blk
    return wt


# fetch groups: (group key list of layout keys) in consumption order
def _fetch_groups():
    gs = []
    gs.append(("near0", [("near", 0, kt) for kt in _JS[0]["kts"]]))
    gs.append(("near1", [("near", 1, kt) for kt in _JS[1]["kts"]]))
    for i in range(_NSTEP):
        gs.append((f"ch{i}",
                   [(f"chain_{pc}", i) for pc, _k in _step_pieces(i)]))
        j = i + 2
        gs.append((f"out{j}",
                   [("near", j, kt) for kt in _JS[j]["kts"]] + [("far", j)]))
    return gs


_FETCH = _fetch_groups()
_WMAX = max(sum(_LAY[k][1] for k in keys) for _, keys in _FETCH)


# ------------------------------------------------------------- device ----

def _build_device_kernel():
    import concourse.mybir as mybir
    from concourse import bacc
    from concourse.tile import TileContext

    f32 = mybir.dt.float32
    bf16 = mybir.dt.bfloat16

    nc = bacc.Bacc()
    xT = nc.dram_tensor("xT", [N, BS], bf16, kind="ExternalInput")
    wt = nc.dram_tensor("wt", [P, _TOTC], bf16, kind="ExternalInput")
    # host-packed chain g0 stacks: step i at free block i
    xga = nc.dram_tensor("xga", [P, _NSTEP * BS], bf16, kind="ExternalInput")
    xgb = nc.dram_tensor("xgb", [16, _NSTEP * BS], bf16, kind="ExternalInput")
    outT = nc.dram_tensor("outT", [N, BS], bf16, kind="ExternalOutput")

    XCH = [4, 4, 4, 4, 2]   # x k-tiles merged into 5 SBUF tiles
    WSIZES = [3, 7, 6, 6, 6, 6]  # w stream chunk sizes (first small)

    with TileContext(nc) as tc:
        with (
            tc.tile_pool(name="xpool", bufs=1) as xpool,
            tc.tile_pool(name="spool", bufs=1) as spool,
            tc.tile_pool(name="wpool", bufs=1) as wpool,
            tc.tile_pool(name="opool", bufs=3) as opool,
            tc.tile_pool(name="pso", bufs=2, space="PSUM") as pso,
            tc.tile_pool(name="pss", bufs=2, space="PSUM") as pss,
        ):
            # Engine / ring roles:
            #   sync  (SP HWDGE): all input loads (x, xg, w)
            #   gpsimd (SWDGE):   output stores
            #   scalar/vector:    PSUM->SBUF copies, alternating
            xch = [xpool.tile([P, nk * BS], bf16, tag=f"x{ci}", name=f"x{ci}")
                   for ci, nk in enumerate(XCH)]
            kt0_of = [0, 4, 8, 12, 16]

            def issue_xk(kt):
                ci = min(kt // 4, 4)
                kk = kt - kt0_of[ci]
                nc.sync.dma_start(out=xch[ci][:, kk * BS:(kk + 1) * BS],
                                  in_=xT[P * kt:P * kt + P, :])

            def x_ap(kt, c0, c1):
                ci = min(kt // 4, 4)
                off = (kt - kt0_of[ci]) * BS
                return xch[ci][:, off + c0:off + c1]

            # chain g0 stacks: ga in 4 chunks (4 steps each), gb whole
            gach = [xpool.tile([P, 4 * BS], bf16, tag=f"ga{q}", name=f"ga{q}")
                    for q in range(4)]
            gb = xpool.tile([16, _NSTEP * BS], bf16, tag="gb", name="gb")

            # operator stream: merged chunk DMAs, consumption order
            wslot = {}
            wchunks = []
            nchunks = len(WSIZES)
            woff = [0]
            for s in WSIZES:
                woff.append(woff[-1] + s)
            assert woff[-1] == len(_FETCH)
            for ci in range(nchunks):
                grp = _FETCH[woff[ci]:woff[ci + 1]]
                keys = [k for _, ks in grp for k in ks]
                o0 = _LAY[keys[0]][0]
                cols = sum(_LAY[k][1] for k in keys)
                wtile = wpool.tile([P, 3584], bf16, tag=f"w{ci}",
                                   name=f"w{ci}")
                wchunks.append((wtile, o0, cols))
                for k in keys:
                    wslot[k] = (wtile, _LAY[k][0] - o0)

            def issue_w(ci):
                wtile, o0, cols = wchunks[ci]
                nc.sync.dma_start(out=wtile[:, :cols],
                                  in_=wt[:, o0:o0 + cols])

            def w_ap(key, kk):
                wtile, o = wslot[key]
                m = 88 if key[0].startswith("chain") else P
                return wtile[0:kk, o:o + m]

            # state tiles
            st = [spool.tile([104, BS], bf16, tag=f"s{i}", name=f"s{i}")
                  for i in range(_NSTEP)]

            def cp_state(dst, src):
                nc.scalar.copy(dst, src)

            def cp_out(dst, src):
                nc.vector.tensor_copy(dst, src)

            def out_group(j):
                d = _JS[j]
                items = [("near", kt) for kt in d["kts"]]
                if d["cut"] is not None:
                    items.append(("far", None))
                ps = pso.tile([P, BS], f32, tag="o", name=f"ps{j}")
                for it, (kind, kt) in enumerate(items):
                    first, last = it == 0, it == len(items) - 1
                    for m in range(NM):
                        if kind == "near":
                            lhsT = w_ap(("near", j, kt), P)
                            rhs = x_ap(kt, m * MW, (m + 1) * MW)
                        else:
                            lhsT = w_ap(("far", j), 88)
                            rhs = st[j - 2][0:88, m * MW:(m + 1) * MW]
                        nc.tensor.matmul(ps[:, m * MW:(m + 1) * MW],
                                         lhsT=lhsT, rhs=rhs,
                                         start=first, stop=last)
                oc = opool.tile([P, BS], bf16, tag="o", name=f"oc{j}")
                if j >= 16:  # tail tiles: parallel half copies + stores
                    nc.vector.tensor_copy(oc[:, 0:MW], ps[:, 0:MW])
                    nc.scalar.copy(oc[:, MW:BS], ps[:, MW:BS])
                    nc.gpsimd.dma_start(out=outT[P * j:P * j + P, 0:MW],
                                        in_=oc[:, 0:MW])
                    nc.sync.dma_start(out=outT[P * j:P * j + P, MW:BS],
                                      in_=oc[:, MW:BS])
                else:
                    cp_out(oc[:], ps[:])
                    nc.gpsimd.dma_start(out=outT[P * j:P * j + P, :],
                                        in_=oc[:])

            def chain_step(i):
                ps = pss.tile([88, BS], f32, tag="s", name=f"pss{i}")
                pieces = _step_pieces(i)
                for it, (pc, kdim) in enumerate(pieces):
                    first, last = it == 0, it == len(pieces) - 1
                    if pc == "ga":
                        rt, base = gach[i // 4], (i % 4) * BS
                    elif pc == "gb":
                        rt, base = gb, i * BS
                    else:
                        rt, base = st[i - 1], 0
                    for m in range(NM):
                        nc.tensor.matmul(
                            ps[:, m * MW:(m + 1) * MW],
                            lhsT=w_ap((f"chain_{pc}", i), kdim),
                            rhs=rt[0:kdim, base + m * MW:base + (m + 1) * MW],
                            start=first, stop=last)
                if i == _NSTEP - 1:  # tail-critical: parallel halves
                    nc.scalar.copy(st[i][0:88, 0:MW], ps[:, 0:MW])
                    nc.vector.tensor_copy(st[i][0:88, MW:BS], ps[:, MW:BS])
                else:
                    cp_state(st[i][0:88, :], ps[:])

            # ---------------- emission ----------------
            # PE warm-up: un-throttle HAM during the initial DMA window
            wu = spool.tile([P, P], bf16, tag="warm", name="warm")
            nc.scalar.memzero(wu[:])
            pwu = pso.tile([P, BS], f32, tag="o", name="pswarm")
            for _ in range(17):
                nc.tensor.matmul(pwu[:, 0:64], lhsT=wu[:], rhs=wu[:, 0:64],
                                 start=True, stop=True)

            issue_xk(0)
            issue_xk(1)
            issue_w(0)
            nc.sync.dma_start(out=gach[0][:],
                              in_=xga[:, 0:4 * BS])
            nc.sync.dma_start(out=gb[:, 0:BS], in_=xgb[:, 0:BS])
            issue_xk(2)
            issue_xk(3)
            issue_w(1)
            nc.sync.dma_start(out=gach[1][:],
                              in_=xga[:, 4 * BS:8 * BS])

            def issue_ov(i):
                ov = max(_step_ng(i) - P, 0)
                if ov:
                    nc.sync.dma_start(
                        out=st[i - 1][88:88 + ov, :],
                        in_=xgb[0:ov, i * BS:(i + 1) * BS])

            for i in range(1, 6):
                issue_ov(i)
            for t in range(4, NT):
                issue_xk(t)
                if t in (5, 8, 11, 14):
                    ci = 2 + (t - 5) // 3
                    if ci < nchunks:
                        issue_w(ci)
                if t == 5:
                    nc.sync.dma_start(out=gach[2][:],
                                      in_=xga[:, 8 * BS:12 * BS])
                    for i in range(6, 11):
                        issue_ov(i)
                if t == 8:
                    nc.sync.dma_start(out=gach[3][:],
                                      in_=xga[:, 12 * BS:16 * BS])
                    for i in range(11, _NSTEP):
                        issue_ov(i)

            out_group(0)
            out_group(1)
            chain_step(0)
            for i in range(1, _NSTEP):
                chain_step(i)
                out_group(i + 1)
            out_group(17)

    if not nc.is_finalized():
        nc.finalize()
    return nc


# -------------------------------------------------------------- driver ----

def kernel(inputs: np.ndarray, weights: np.ndarray) -> np.ndarray:
    import ml_dtypes
    from concourse.bass_utils import run_bass_kernel_spmd

    inputs = np.ascontiguousarray(inputs, dtype=np.float32)
    weights = np.ascontiguousarray(weights, dtype=np.float32)

    ops = _build_operators(weights)
    wt_packed = np.ascontiguousarray(_pack_ops(ops)).astype(ml_dtypes.bfloat16)

    # x-major per-sample flatten, then transpose so grid index leads
    xP = inputs.reshape(B, SIZE, SIZE).transpose(0, 2, 1).reshape(B, N)

    nc = _build_device_kernel()
    in_maps = []
    for c in range(NCORES):
        xc = np.ascontiguousarray(xP[c * BS:(c + 1) * BS].T)  # (N, BS) fp32
        xga = np.zeros((P, _NSTEP * BS), dtype=np.float32)
        xgb = np.zeros((16, _NSTEP * BS), dtype=np.float32)
        for i in range(_NSTEP):
            c0, _c1 = _step_cols(i)
            ng = _step_ng(i)
            r0 = SIZE * c0
            ka = min(ng, P)
            xga[:ka, i * BS:(i + 1) * BS] = xc[r0:r0 + ka]
            if ng > P:
                xgb[:ng - P, i * BS:(i + 1) * BS] = xc[r0 + P:r0 + ng]
        in_maps.append({
            "xT": xc.astype(ml_dtypes.bfloat16),
            "wt": wt_packed,
            "xga": xga.astype(ml_dtypes.bfloat16),
            "xgb": xgb.astype(ml_dtypes.bfloat16),
        })
    trace = bool(int(os.environ.get("KERNEL_TRACE", "0")))
    res = run_bass_kernel_spmd(
        nc, in_maps, core_ids=list(range(NCORES)), trace=trace
    )
    if trace and res.exec_time_ns is not None:
        print(f"HW exec time: {res.exec_time_ns} ns")
        if res.instructions_and_trace is not None:
            print(f"trace: {res.instructions_and_trace[1]}")

    outP = np.concatenate(
        [res.results[c]["outT"].astype(np.float32).T for c in range(NCORES)],
        axis=0,
    )
    return np.ascontiguousarray(
        outP.reshape(B, SIZE, SIZE).transpose(0, 2, 1).reshape(B, N)
    )
